# revision 29
# baseline (speedup 1.0000x reference)
"""HGT layer (2 node types, 2 relations) on 8 Trainium2 cores.

Strategy (dst-sharded, fully fused single pass):
  - Each core owns 12500 destination nodes of each type. Edges are
    partitioned by destination shard on the host and sorted into groups
    of 128 consecutive destination nodes, padded to T*128 slots.
  - bf16 datapath: gathers, one-hot matrices, and every matmul operand
    are bf16 (PE runs 1 cycle/row vs 4 for fp32; gather HBM traffic
    halves).  PSUM accumulation and the layernorm chain stay fp32.
  - x_dst arrives pre-transposed from the host (direct lhsT for the
    Q/skip matmuls); the per-edge one-hot matrices are also host-built
    in both orientations and simply DMA-loaded, so the vector engine
    never constructs them.  Gathered source rows are transposed on the
    tensor engine (128x128 bf16 transposes).
  - The scalar engine only ever uses {Copy, Relu, Square, Exp, Ln} —
    all within the single `natural_log_exp_and_others` activation-table
    set (the compile patches the table map so the load-inserter can
    prove this; a pre-loop warm activation makes the in-loop load
    hoistable) — so no ACT_TABLE_LOAD reloads occur in steady state.
    rsqrt (layernorm) and the attention-denominator reciprocal are both
    computed as exp(-a*ln(x)) on the scalar engine.
  - The per-edge exp'd attention logits are written into 8 extra
    columns appended to the weighted-V tile, so the softmax denominator
    falls out of the same one-hot aggregation matmul stream as the
    numerator (transposed: den^T [h, d]); a tiny one-hot head-replication
    matmul expands 1/den back to feature rows.
  - Only one PSUM accumulation chain is ever open per PSUM bank, and a
    chain's matmuls are always consecutive on the tensor engine.
  - The group loop is unrolled 49x inside tc.For_i: the per-iteration
    all-engine barrier amortizes away and the Tile scheduler pipelines
    the unrolled groups across engines (PSUM tag bufs are chosen so
    cross-group waits land on weak positions of the dependency chain).
  - The per-head attention scale (SCALE * sigmoid(mu_h)) is folded into
    Wq on the host.  When the LN affine params are identity and the skip
    bias is zero (always true for this model's inputs), those ops are
    compiled out.
"""

import numpy as np
import ml_dtypes

import concourse.bacc as bacc
import concourse.bass as bass
import concourse.mybir as mybir
import concourse.tile as tile
from concourse.bass import ds
from concourse.masks import make_identity

N = 100000
D = 256
H = 8
DH = 32
M = 8            # cores
NSH = N // M     # 12500 dst rows per core per type
G = 98           # dst groups of 128 per core (98*128 = 12544)
NPAD = G * 128   # 12544
UNROLL = 49      # dst groups per hardware-loop iteration
EPS = 1e-5
DA = D + H       # wV tile width with appended exp'd logits
F32 = mybir.dt.float32
BF16 = mybir.dt.bfloat16
I32 = mybir.dt.int32
AF = mybir.ActivationFunctionType
OP = mybir.AluOpType
NPBF = ml_dtypes.bfloat16

ACT_SET = "natural_log_exp_and_others"
ACT_FNS = {AF.Copy, AF.Exp, AF.Relu, AF.Square, AF.Ln}


# ----------------------------------------------------------------- host prep

def _pack_edges(src, dst, T):
    """Partition edges by dst shard, group by 128 consecutive dsts, pad to
    T*128 slots per group.  Returns src_idx [M, NPAD, T] int32 and
    dstl [M, NPAD, T] bf16 (dst-local-in-group; 999.0 for padding).
    Slot s of group g maps to SBUF (partition p = s % 128, column t = s // 128),
    i.e. row g*128 + p, col t of the packed array."""
    order = np.argsort(dst, kind="stable")
    s_sorted = src[order].astype(np.int64)
    d_sorted = dst[order].astype(np.int64)

    core = d_sorted // NSH
    local = d_sorted - core * NSH
    grp = local // 128
    dloc = local - grp * 128
    key = core * G + grp
    # rank of each edge within its (core, group)
    first = np.r_[0, np.flatnonzero(np.diff(key)) + 1]
    starts = np.zeros(len(key), dtype=np.int64)
    starts[first] = first
    starts = np.maximum.accumulate(starts)
    slot = np.arange(len(key), dtype=np.int64) - starts

    maxslot = int(slot.max()) if len(slot) else 0
    assert maxslot < T * 128, f"edge capacity exceeded: {maxslot + 1} > {T * 128}"

    src_arr = np.zeros((M * G, T * 128), dtype=np.int32)
    dst_arr = np.full((M * G, T * 128), 999.0, dtype=np.float32)
    src_arr[key, slot] = s_sorted
    dst_arr[key, slot] = dloc
    # [MG, T, 128] -> [MG, 128, T] -> [M, NPAD, T]
    src_arr = src_arr.reshape(M * G, T, 128).transpose(0, 2, 1)
    dst_arr = dst_arr.reshape(M * G, T, 128).transpose(0, 2, 1)  # [MG, e, T]
    # one-hot matrices in both orientations, built host-side:
    #   oT[e, t, d] = (dstl[e, t] == d)   (edge-major, rhs/lhsT of aggregation)
    #   od[d, t, e] = (dstl[e, t] == d)   (dst-major, lhsT of q-expansion)
    eq = dst_arr[:, :, :, None] == np.arange(128, dtype=np.float32)
    oT_h = eq.astype(NPBF).reshape(M, NPAD, T * 128)
    od_h = eq.transpose(0, 3, 2, 1).astype(NPBF).reshape(M, NPAD, T * 128)
    return (src_arr.reshape(M, NPAD, T).copy(),
            oT_h.copy(), od_h.copy())


def _edge_capacity(dst):
    d = np.sort(dst.astype(np.int64))
    core = d // NSH
    grp = (d - core * NSH) // 128
    key = core * G + grp
    _, counts = np.unique(key, return_counts=True)
    return int(counts.max())


def _shard_rows_T(x):
    """[N, D] -> [M, NPAD, 2, 128] bf16, zero padded, per-group transposed:
    out[m, g*128 + p, c, e] = x[m*NSH + g*128 + e, c*128 + p]."""
    out = np.zeros((M, G, 128, 2, 128), dtype=NPBF)
    for m in range(M):
        blk = np.zeros((NPAD, D), dtype=np.float32)
        blk[:NSH] = x[m * NSH:(m + 1) * NSH]
        # [G, 128e, 2c, 128p] -> [G, p, c, e]
        out[m] = blk.reshape(G, 128, 2, 128).transpose(0, 3, 2, 1).astype(NPBF)
    return out.reshape(M, NPAD, 2, 128)


# ------------------------------------------------------------- bass program

def _patched_act_tables(orig_fn):
    """Return a wrapper that strips this kernel's activation functions from
    every table set except ACT_SET, so the table-load inserter maps them all
    to one set (positions/indices of the sets are preserved)."""
    def wrapper(arch):
        t = orig_fn(arch)
        if ACT_SET in t:
            for name in t:
                if name != ACT_SET:
                    t[name] = t[name] - ACT_FNS
        return t
    return wrapper


def build_program(T, use_affine, use_bias, nfull=N, npad=NPAD):
    nc = bacc.Bacc("TRN2", target_bir_lowering=False, debug=False)

    def drt(name, shape, dtype=F32, kind="ExternalInput"):
        return nc.dram_tensor(name, shape, dtype, kind=kind)

    xa_full = drt("xa_full", [nfull, D], BF16)
    xb_full = drt("xb_full", [nfull, D], BF16)
    xa_dstT = drt("xa_dstT", [npad, 2, 128], BF16)
    xb_dstT = drt("xb_dstT", [npad, 2, 128], BF16)
    repl_row = drt("repl_row", [H, D], BF16)    # repl[h, f] = (f//DH == h)

    rels = []
    for r in ("ab", "ba"):
        rel = dict(
            name=r,
            src=drt(f"src_{r}", [npad, T], I32),
            oT=drt(f"oT_{r}", [npad, T * 128], BF16),
            od=drt(f"od_{r}", [npad, T * 128], BF16),
            wq=drt(f"wq_{r}", [D, D], BF16),
            wk=drt(f"wk_{r}", [D, D], BF16),
            wv=drt(f"wv_{r}", [D, D], BF16),
            wmsg=drt(f"wmsg_{r}", [D, D], BF16),
            wskip=drt(f"wskip_{r}", [D, D], BF16),
            out=drt(f"out_{r}", [npad, D], kind="ExternalOutput"),
        )
        if use_bias:
            rel["bskip"] = drt(f"bskip_{r}", [1, D], BF16)
        if use_affine:
            rel["gln"] = drt(f"gln_{r}", [128, D])
            rel["bln"] = drt(f"bln_{r}", [128, D])
        rels.append(rel)
    rels[0]["xfull"] = xa_full   # ab: src type a
    rels[0]["xdstT"] = xb_dstT   # ab: dst type b
    rels[1]["xfull"] = xb_full
    rels[1]["xdstT"] = xa_dstT

    with tile.TileContext(nc) as tc:
        with (
            tc.tile_pool(name="const", bufs=1) as cp,
            tc.tile_pool(name="sbuf", bufs=2) as sp,
            tc.tile_pool(name="psum", bufs=1, space="PSUM") as pp,
        ):
            ident = cp.tile([128, 128], BF16)
            make_identity(nc, ident[:])
            repl = cp.tile([H, 2, 128], BF16)
            for c in range(2):
                nc.sync.dma_start(out=repl[:, c, :],
                                  in_=repl_row[:, c * 128:(c + 1) * 128])
            if use_bias:
                ones1 = cp.tile([1, 128], BF16)
                nc.gpsimd.memset(ones1[:], 1.0)
            epsd = cp.tile([H, 1], F32)
            nc.gpsimd.memset(epsd[:], 1e-30)
            epsv = cp.tile([128, 1], F32)
            nc.gpsimd.memset(epsv[:], EPS)
            # prime the activation-table set before the loops so the
            # in-loop table-load check can hoist (preheader = loaded)
            warm = cp.tile([1, 1], F32)
            nc.scalar.activation(warm[:], epsv[0:1, :], AF.Exp)

            for rel in rels:
                # --- static per-relation weights
                wq = cp.tile([128, 2, D], BF16, tag="wq")
                wk = cp.tile([128, 2, D], BF16, tag="wk")
                wv = cp.tile([128, 2, D], BF16, tag="wv")
                wmsg = cp.tile([128, 2, D], BF16, tag="wmsg")
                wskip = cp.tile([128, 2, D], BF16, tag="wskip")
                for c in range(2):
                    nc.sync.dma_start(out=wq[:, c, :], in_=rel["wq"][c * 128:(c + 1) * 128, :])
                    nc.sync.dma_start(out=wk[:, c, :], in_=rel["wk"][c * 128:(c + 1) * 128, :])
                    nc.sync.dma_start(out=wv[:, c, :], in_=rel["wv"][c * 128:(c + 1) * 128, :])
                    nc.sync.dma_start(out=wmsg[:, c, :], in_=rel["wmsg"][c * 128:(c + 1) * 128, :])
                    nc.sync.dma_start(out=wskip[:, c, :], in_=rel["wskip"][c * 128:(c + 1) * 128, :])
                if use_bias:
                    bskip = cp.tile([1, D], BF16, tag="bskip")
                    nc.sync.dma_start(out=bskip[:], in_=rel["bskip"][:])
                if use_affine:
                    gln = cp.tile([128, D], F32, tag="gln")
                    bln = cp.tile([128, D], F32, tag="bln")
                    nc.sync.dma_start(out=gln[:], in_=rel["gln"][:])
                    nc.sync.dma_start(out=bln[:], in_=rel["bln"][:])

                xfull, xdstT, srcd, oTd, odd, outd = (
                    rel["xfull"], rel["xdstT"], rel["src"], rel["oT"],
                    rel["od"], rel["out"])

                with tc.For_i(0, npad, 128 * UNROLL) as g0:
                  for u in range(UNROLL):
                    gg = g0 + 128 * u
                    # ---- loads (x_dst arrives pre-transposed)
                    xdT = sp.tile([128, 2, 128], BF16, tag="xdT", bufs=3)
                    nc.sync.dma_start(out=xdT[:], in_=xdstT[ds(gg, 128), :, :])
                    sidx = sp.tile([128, T], I32, tag="sidx", bufs=3)
                    nc.sync.dma_start(out=sidx[:], in_=srcd[ds(gg, 128), :])
                    xg = sp.tile([128, T, D], BF16, tag="xg", bufs=3)
                    for t in range(T):
                        nc.gpsimd.indirect_dma_start(
                            out=xg[:, t, :], out_offset=None,
                            in_=xfull[:],
                            in_offset=bass.IndirectOffsetOnAxis(
                                ap=sidx[:, t:t + 1], axis=0),
                        )

                    # ---- Q projection
                    qg_ps = pp.tile([128, D], F32, tag="qg", bufs=1)
                    for c in range(2):
                        nc.tensor.matmul(out=qg_ps[:], lhsT=xdT[:, c, :],
                                         rhs=wq[:, c, :],
                                         start=(c == 0), stop=(c == 1))
                    qg = sp.tile([128, D], BF16, tag="qg")
                    nc.scalar.copy(qg[:], qg_ps[:])

                    # ---- one-hot (edge -> dst-local), host-built, both
                    # orientations loaded directly
                    oT = sp.tile([128, T, 128], BF16, tag="oT", bufs=3)
                    nc.sync.dma_start(
                        out=oT[:],
                        in_=oTd[ds(gg, 128), :].rearrange(
                            "p (t d) -> p t d", d=128))
                    od = sp.tile([128, T, 128], BF16, tag="odsb", bufs=3)
                    nc.sync.dma_start(
                        out=od[:],
                        in_=odd[ds(gg, 128), :].rearrange(
                            "p (t d) -> p t d", d=128))

                    # ---- gathered x^T (tensor-engine transposes)
                    xgT_ps = pp.tile([128, T, 2, 128], BF16, tag="xgT")
                    for t in range(T):
                        for c in range(2):
                            nc.tensor.transpose(
                                out=xgT_ps[:, t, c, :],
                                in_=xg[:, t, c * 128:(c + 1) * 128],
                                identity=ident[:])
                    xgT = sp.tile([128, T, 2, 128], BF16, tag="xgTsb")
                    hT = T // 2
                    nc.vector.tensor_copy(xgT[:, :hT], xgT_ps[:, :hT])
                    nc.scalar.copy(xgT[:, hT:], xgT_ps[:, hT:])

                    # ---- per-edge K / q-expand / V; attention logits
                    qkm = sp.tile([128, T, D], BF16, tag="qkm")
                    k_sb = sp.tile([128, T, D], BF16, tag="k_sb")
                    wV = sp.tile([128, T, DA], BF16, tag="wV")
                    npair = (T + 1) // 2
                    # K for all T in one 2-bank PSUM tile: no k-bank stall
                    # mid-stream, so the PE's opening matmul run is long
                    # enough to trigger the clock ramp; one batched k copy.
                    k_ps = pp.tile([128, T, D], F32, tag="k")
                    for t in range(T):
                        for c in range(2):
                            nc.tensor.matmul(
                                out=k_ps[:, t, :], lhsT=xgT[:, t, c, :],
                                rhs=wk[:, c, :], start=(c == 0), stop=(c == 1))
                    nc.scalar.copy(k_sb[:], k_ps[:])
                    for p in range(npair):
                        t0, t1 = 2 * p, min(2 * p + 2, T)
                        tw = t1 - t0
                        qe_ps = pp.tile([128, 2, D], F32, tag="qe",
                                        bufs=1, name="qe_ps")
                        for t in range(t0, t1):
                            nc.tensor.matmul(
                                out=qe_ps[:, t - t0, :], lhsT=od[:, t, :],
                                rhs=qg[:], start=True, stop=True)
                        nc.vector.tensor_tensor(
                            out=qkm[:, t0:t1, :], in0=qe_ps[:, :tw, :],
                            in1=k_sb[:, t0:t1, :], op=OP.mult)

                    # exp'd logits land in the extra columns of wV, so the
                    # denominator aggregates in the same matmul stream
                    attn = sp.tile([128, T * H], F32, tag="attn")
                    ae = wV[:, :, D:DA]
                    for p in range(npair):
                        t0, t1 = 2 * p, min(2 * p + 2, T)
                        nc.vector.tensor_reduce(
                            out=attn[:, t0 * H:t1 * H],
                            in_=qkm[:, t0:t1].rearrange(
                                "p t (h j) -> p (t h) j", j=DH),
                            axis=mybir.AxisListType.X, op=OP.add)
                        nc.scalar.activation(
                            ae[:, t0:t1, :],
                            attn[:, t0 * H:t1 * H].rearrange(
                                "p (t h) -> p t h", h=H), AF.Exp)

                    # ---- V projection + weighted rows, in pairs
                    for p in range(npair):
                        t0, t1 = 2 * p, min(2 * p + 2, T)
                        tw = t1 - t0
                        v_ps = pp.tile([128, 2, D], F32, tag="v", bufs=1)
                        for t in range(t0, t1):
                            for c in range(2):
                                nc.tensor.matmul(
                                    out=v_ps[:, t - t0, :], lhsT=xgT[:, t, c, :],
                                    rhs=wv[:, c, :], start=(c == 0), stop=(c == 1))
                        nc.vector.tensor_tensor(
                            out=wV[:, t0:t1, 0:D].rearrange(
                                "p t (h j) -> p t h j", j=DH),
                            in0=ae[:, t0:t1, :, None].to_broadcast(
                                [128, tw, H, DH]),
                            in1=v_ps[:, :tw].rearrange(
                                "p t (h j) -> p t h j", j=DH),
                            op=OP.mult)

                    # ---- one-hot aggregation: numerator + denominator^T.
                    # One open accumulation chain per PSUM bank at a time;
                    # agg chunks and den share a bank, chains sequential.
                    accum_ps = pp.tile([128, 384], F32, tag="accum", bufs=1)
                    for c in range(2):
                        for t in range(T):
                            nc.tensor.matmul(
                                out=accum_ps[:, c * 128:(c + 1) * 128],
                                lhsT=wV[:, t, c * 128:(c + 1) * 128],
                                rhs=oT[:, t, :],
                                start=(t == 0), stop=(t == T - 1))
                    for t in range(T):
                        nc.tensor.matmul(
                            out=accum_ps[0:H, 256:384], lhsT=wV[:, t, D:DA],
                            rhs=oT[:, t, :],
                            start=(t == 0), stop=(t == T - 1))

                    # ---- 1/den^T = exp(-ln(den+eps)), expanded via repl
                    lnd = sp.tile([H, 128], F32, tag="lnd")
                    nc.scalar.activation(lnd[:], accum_ps[0:H, 256:384],
                                         AF.Ln, bias=epsd[:, :1])
                    dinvb = sp.tile([H, 128], BF16, tag="dinvb")
                    nc.scalar.activation(dinvb[:], lnd[:], AF.Exp, scale=-1.0)
                    R_ps = pp.tile([128, D], F32, tag="ry", bufs=1)
                    for c in range(2):
                        nc.tensor.matmul(out=R_ps[:, c * 128:(c + 1) * 128],
                                         lhsT=repl[:, c, :], rhs=dinvb[:],
                                         start=(c == 0), stop=(c == 1))
                    Rsb = sp.tile([128, 2, 128], BF16, tag="Rsb")
                    nc.scalar.copy(Rsb[:], R_ps[:].rearrange("p (c d) -> p c d", d=128))
                    aggT = sp.tile([128, 2, 128], BF16, tag="aggT")
                    nc.vector.tensor_tensor(
                        out=aggT[:],
                        in0=accum_ps[:, 0:256].rearrange("p (c d) -> p c d", d=128),
                        in1=Rsb[:], op=OP.mult)

                    # ---- y = aggT.T @ Wmsg + x @ Wskip (+ bskip); relu; LN
                    y_ps = pp.tile([128, D], F32, tag="ry", bufs=1)
                    if use_bias:
                        nc.tensor.matmul(out=y_ps[:], lhsT=ones1[:], rhs=bskip[:],
                                         start=True, stop=False)
                    for c in range(2):
                        nc.tensor.matmul(out=y_ps[:], lhsT=aggT[:, c, :],
                                         rhs=wmsg[:, c, :],
                                         start=(not use_bias and c == 0),
                                         stop=False)
                    for c in range(2):
                        nc.tensor.matmul(out=y_ps[:], lhsT=xdT[:, c, :],
                                         rhs=wskip[:, c, :], start=False,
                                         stop=(c == 1))
                    zr = sp.tile([128, D], F32, tag="zr")
                    msum = sp.tile([128, 1], F32, tag="msum")
                    nc.scalar.activation(zr[:], y_ps[:], AF.Relu,
                                         accum_out=msum[:, :1])
                    negm = sp.tile([128, 1], F32, tag="negm")
                    nc.vector.tensor_scalar(out=negm[:], in0=msum[:],
                                            scalar1=-1.0 / D, scalar2=None,
                                            op0=OP.mult)
                    sqd = sp.tile([128, D], F32, tag="sqd")
                    vs = sp.tile([128, 1], F32, tag="vs")
                    nc.scalar.activation(sqd[:], zr[:], AF.Square,
                                         bias=negm[:, :1],
                                         accum_out=vs[:, :1])
                    lnv = sp.tile([128, 1], F32, tag="lnv")
                    nc.scalar.activation(lnv[:], vs[:], AF.Ln,
                                         scale=1.0 / D, bias=epsv[:, :1])
                    rstd = sp.tile([128, 1], F32, tag="rstd")
                    nc.scalar.activation(rstd[:], lnv[:], AF.Exp, scale=-0.5)
                    fin = sp.tile([128, D], F32, tag="fin")
                    nc.vector.tensor_scalar(out=fin[:], in0=zr[:],
                                            scalar1=negm[:, :1],
                                            scalar2=rstd[:, :1],
                                            op0=OP.add, op1=OP.mult)
                    if use_affine:
                        xg2 = sp.tile([128, D], F32, tag="xg2")
                        nc.vector.tensor_tensor(out=xg2[:], in0=fin[:],
                                                in1=gln[:], op=OP.mult)
                        nc.vector.tensor_tensor(out=fin[:], in0=xg2[:],
                                                in1=bln[:], op=OP.add)
                    nc.scalar.dma_start(out=outd[ds(gg, 128), :], in_=fin[:])
    orig = bacc.get_activation_tables
    bacc.get_activation_tables = _patched_act_tables(orig)
    try:
        nc.compile()
    finally:
        bacc.get_activation_tables = orig
    return nc


# ------------------------------------------------------------------- driver

def _sigmoid(x):
    return 1.0 / (1.0 + np.exp(-x))


TRACE = False
LAST = None


def kernel(x_a, x_b, Wq_a, Wk_a, Wv_a, Wq_b, Wk_b, Wv_b,
           Wskip_a_w, Wskip_a_b, Wskip_b_w, Wskip_b_b,
           g_a, b_a, g_b, b_b, mu_ab, Wmsg_ab, mu_ba, Wmsg_ba,
           ei_ab, ei_ba):
    from concourse.bass_utils import run_bass_kernel_spmd

    x_a = np.asarray(x_a, np.float32)
    x_b = np.asarray(x_b, np.float32)
    SCALE = DH ** -0.5

    cap = max(_edge_capacity(np.asarray(ei_ab[1])),
              _edge_capacity(np.asarray(ei_ba[1])))
    T = max(2, -(-cap // 128))

    src_ab, oT_ab, od_ab = _pack_edges(
        np.asarray(ei_ab[0]), np.asarray(ei_ab[1]), T)
    src_ba, oT_ba, od_ba = _pack_edges(
        np.asarray(ei_ba[0]), np.asarray(ei_ba[1]), T)

    xa_dstT = _shard_rows_T(x_a)
    xb_dstT = _shard_rows_T(x_b)

    use_affine = not (
        np.allclose(np.asarray(g_a), 1.0) and np.allclose(np.asarray(b_a), 0.0)
        and np.allclose(np.asarray(g_b), 1.0) and np.allclose(np.asarray(b_b), 0.0))
    use_bias = bool(np.any(np.asarray(Wskip_a_b)) or np.any(np.asarray(Wskip_b_b)))

    def fold_q(Wq, mu):
        s = (SCALE * _sigmoid(np.asarray(mu, np.float64))).astype(np.float32)
        return (np.asarray(Wq, np.float32) * np.repeat(s, DH)[None, :]).astype(NPBF)

    bf = lambda v: np.asarray(v, np.float32).astype(NPBF)
    bc = lambda v: np.broadcast_to(np.asarray(v, np.float32)[None, :], (128, D)).copy()
    repl_row = (np.arange(D)[None, :] // DH ==
                np.arange(H)[:, None]).astype(NPBF).copy()

    shared = {
        "xa_full": x_a.astype(NPBF), "xb_full": x_b.astype(NPBF),
        "repl_row": repl_row,
        # relation ab: src a -> dst b (out_b)
        "wq_ab": fold_q(Wq_b, mu_ab), "wk_ab": bf(Wk_a), "wv_ab": bf(Wv_a),
        "wmsg_ab": bf(Wmsg_ab), "wskip_ab": bf(Wskip_b_w),
        # relation ba: src b -> dst a (out_a)
        "wq_ba": fold_q(Wq_a, mu_ba), "wk_ba": bf(Wk_b), "wv_ba": bf(Wv_b),
        "wmsg_ba": bf(Wmsg_ba), "wskip_ba": bf(Wskip_a_w),
    }
    if use_bias:
        shared["bskip_ab"] = bf(Wskip_b_b).reshape(1, D)
        shared["bskip_ba"] = bf(Wskip_a_b).reshape(1, D)
    if use_affine:
        shared["gln_ab"] = bc(g_b)
        shared["bln_ab"] = bc(b_b)
        shared["gln_ba"] = bc(g_a)
        shared["bln_ba"] = bc(b_a)

    in_maps = []
    for m in range(M):
        im = dict(shared)
        im["xa_dstT"] = xa_dstT[m]
        im["xb_dstT"] = xb_dstT[m]
        im["src_ab"] = src_ab[m]
        im["oT_ab"] = oT_ab[m]
        im["od_ab"] = od_ab[m]
        im["src_ba"] = src_ba[m]
        im["oT_ba"] = oT_ba[m]
        im["od_ba"] = od_ba[m]
        in_maps.append(im)

    nc = build_program(T, use_affine, use_bias)
    res = run_bass_kernel_spmd(nc, in_maps, list(range(M)), trace=TRACE)
    global LAST
    LAST = res
    out_a = np.empty((N, D), np.float32)
    out_b = np.empty((N, D), np.float32)
    for m in range(M):
        out_b[m * NSH:(m + 1) * NSH] = res.results[m]["out_ab"][:NSH]
        out_a[m * NSH:(m + 1) * NSH] = res.results[m]["out_ba"][:NSH]
    return out_a, out_b


# revision 31
# speedup vs baseline: 1.2622x; 1.2622x over previous
"""HGT layer (2 node types, 2 relations) on 8 Trainium2 cores.

Strategy (dst-sharded, fully fused single pass):
  - Each core owns 12500 destination nodes of each type. Edges are
    partitioned by destination shard on the host and sorted into groups
    of 128 consecutive destination nodes, padded to T*128 slots.
  - bf16 datapath: gathers, one-hot matrices, and every matmul operand
    are bf16 (PE runs 1 cycle/row vs 4 for fp32; gather HBM traffic
    halves).  PSUM accumulation and the layernorm chain stay fp32.
  - x_dst arrives pre-transposed from the host (direct lhsT for the
    Q/skip matmuls); the per-edge one-hot matrices are also host-built
    in both orientations and simply DMA-loaded, so the vector engine
    never constructs them.  Gathered source rows are transposed on the
    tensor engine (128x128 bf16 transposes).
  - The scalar engine only ever uses {Copy, Relu, Square, Exp, Ln} —
    all within the single `natural_log_exp_and_others` activation-table
    set (the compile patches the table map so the load-inserter can
    prove this; a pre-loop warm activation makes the in-loop load
    hoistable) — so no ACT_TABLE_LOAD reloads occur in steady state.
    rsqrt (layernorm) and the attention-denominator reciprocal are both
    computed as exp(-a*ln(x)) on the scalar engine.
  - The per-edge exp'd attention logits are written into 8 extra
    columns appended to the weighted-V tile, so the softmax denominator
    falls out of the same one-hot aggregation matmul stream as the
    numerator (transposed: den^T [h, d]); a tiny one-hot head-replication
    matmul expands 1/den back to feature rows.
  - Only one PSUM accumulation chain is ever open per PSUM bank, and a
    chain's matmuls are always consecutive on the tensor engine.
  - The group loop is unrolled 49x inside tc.For_i: the per-iteration
    all-engine barrier amortizes away and the Tile scheduler pipelines
    the unrolled groups across engines (PSUM tag bufs are chosen so
    cross-group waits land on weak positions of the dependency chain).
  - The per-head attention scale (SCALE * sigmoid(mu_h)) is folded into
    Wq on the host.  When the LN affine params are identity and the skip
    bias is zero (always true for this model's inputs), those ops are
    compiled out.
"""

import numpy as np
import ml_dtypes

import concourse.bacc as bacc
import concourse.bass as bass
import concourse.mybir as mybir
import concourse.tile as tile
from concourse.bass import ds
from concourse.masks import make_identity

N = 100000
D = 256
H = 8
DH = 32
M = 8            # cores
NSH = N // M     # 12500 dst rows per core per type
G = 98           # dst groups of 128 per core (98*128 = 12544)
NPAD = G * 128   # 12544
UNROLL = 49      # dst groups per hardware-loop iteration
EPS = 1e-5
DA = D + H       # wV tile width with appended exp'd logits
F32 = mybir.dt.float32
BF16 = mybir.dt.bfloat16
I32 = mybir.dt.int32
AF = mybir.ActivationFunctionType
OP = mybir.AluOpType
NPBF = ml_dtypes.bfloat16

ACT_SET = "natural_log_exp_and_others"
ACT_FNS = {AF.Copy, AF.Exp, AF.Relu, AF.Square, AF.Ln}


# ----------------------------------------------------------------- host prep

def _pack_edges(src, dst, T):
    """Partition edges by dst shard, group by 128 consecutive dsts, pad to
    T*128 slots per group.  Returns src_idx [M, NPAD, T] int32 and
    dstl [M, NPAD, T] bf16 (dst-local-in-group; 999.0 for padding).
    Slot s of group g maps to SBUF (partition p = s % 128, column t = s // 128),
    i.e. row g*128 + p, col t of the packed array."""
    order = np.argsort(dst, kind="stable")
    s_sorted = src[order].astype(np.int64)
    d_sorted = dst[order].astype(np.int64)

    core = d_sorted // NSH
    local = d_sorted - core * NSH
    grp = local // 128
    dloc = local - grp * 128
    key = core * G + grp
    # rank of each edge within its (core, group)
    first = np.r_[0, np.flatnonzero(np.diff(key)) + 1]
    starts = np.zeros(len(key), dtype=np.int64)
    starts[first] = first
    starts = np.maximum.accumulate(starts)
    slot = np.arange(len(key), dtype=np.int64) - starts

    maxslot = int(slot.max()) if len(slot) else 0
    assert maxslot < T * 128, f"edge capacity exceeded: {maxslot + 1} > {T * 128}"

    src_arr = np.zeros((M * G, T * 128), dtype=np.int32)
    dst_arr = np.full((M * G, T * 128), 999.0, dtype=np.float32)
    src_arr[key, slot] = s_sorted
    dst_arr[key, slot] = dloc
    # [MG, T, 128] -> [MG, 128, T] -> [M, NPAD, T]
    src_arr = src_arr.reshape(M * G, T, 128).transpose(0, 2, 1)
    dst_arr = dst_arr.reshape(M * G, T, 128).transpose(0, 2, 1)  # [MG, e, T]
    # one-hot matrices in both orientations, built host-side:
    #   oT[e, t, d] = (dstl[e, t] == d)   (edge-major, rhs/lhsT of aggregation)
    #   od[d, t, e] = (dstl[e, t] == d)   (dst-major, lhsT of q-expansion)
    eq = dst_arr[:, :, :, None] == np.arange(128, dtype=np.float32)
    oT_h = eq.astype(NPBF).reshape(M, NPAD, T * 128)
    od_h = eq.transpose(0, 3, 2, 1).astype(NPBF).reshape(M, NPAD, T * 128)
    return (src_arr.reshape(M, NPAD, T).copy(),
            oT_h.copy(), od_h.copy())


def _edge_capacity(dst):
    d = np.sort(dst.astype(np.int64))
    core = d // NSH
    grp = (d - core * NSH) // 128
    key = core * G + grp
    _, counts = np.unique(key, return_counts=True)
    return int(counts.max())


def _shard_rows_T(x):
    """[N, D] -> [M, NPAD, 2, 128] bf16, zero padded, per-group transposed:
    out[m, g*128 + p, c, e] = x[m*NSH + g*128 + e, c*128 + p]."""
    out = np.zeros((M, G, 128, 2, 128), dtype=NPBF)
    for m in range(M):
        blk = np.zeros((NPAD, D), dtype=np.float32)
        blk[:NSH] = x[m * NSH:(m + 1) * NSH]
        # [G, 128e, 2c, 128p] -> [G, p, c, e]
        out[m] = blk.reshape(G, 128, 2, 128).transpose(0, 3, 2, 1).astype(NPBF)
    return out.reshape(M, NPAD, 2, 128)


# ------------------------------------------------------------- bass program

def _patched_act_tables(orig_fn):
    """Return a wrapper that strips this kernel's activation functions from
    every table set except ACT_SET, so the table-load inserter maps them all
    to one set (positions/indices of the sets are preserved)."""
    def wrapper(arch):
        t = orig_fn(arch)
        if ACT_SET in t:
            for name in t:
                if name != ACT_SET:
                    t[name] = t[name] - ACT_FNS
        return t
    return wrapper


def build_program(T, use_affine, use_bias, nfull=N, npad=NPAD):
    nc = bacc.Bacc("TRN2", target_bir_lowering=False, debug=False)

    def drt(name, shape, dtype=F32, kind="ExternalInput"):
        return nc.dram_tensor(name, shape, dtype, kind=kind)

    xa_full = drt("xa_full", [nfull, D], BF16)
    xb_full = drt("xb_full", [nfull, D], BF16)
    xa_dstT = drt("xa_dstT", [npad, 2, 128], BF16)
    xb_dstT = drt("xb_dstT", [npad, 2, 128], BF16)
    repl_row = drt("repl_row", [H, D], BF16)    # repl[h, f] = (f//DH == h)

    rels = []
    for r in ("ab", "ba"):
        rel = dict(
            name=r,
            src=drt(f"src_{r}", [npad, T], I32),
            oT=drt(f"oT_{r}", [npad, T * 128], BF16),
            od=drt(f"od_{r}", [npad, T * 128], BF16),
            wq=drt(f"wq_{r}", [D, D], BF16),
            wk=drt(f"wk_{r}", [D, D], BF16),
            wv=drt(f"wv_{r}", [D, D], BF16),
            wmsg=drt(f"wmsg_{r}", [D, D], BF16),
            wskip=drt(f"wskip_{r}", [D, D], BF16),
            out=drt(f"out_{r}", [npad, D], kind="ExternalOutput"),
        )
        if use_bias:
            rel["bskip"] = drt(f"bskip_{r}", [1, D], BF16)
        if use_affine:
            rel["gln"] = drt(f"gln_{r}", [128, D])
            rel["bln"] = drt(f"bln_{r}", [128, D])
        rels.append(rel)
    rels[0]["xfull"] = xa_full   # ab: src type a
    rels[0]["xdstT"] = xb_dstT   # ab: dst type b
    rels[1]["xfull"] = xb_full
    rels[1]["xdstT"] = xa_dstT

    with tile.TileContext(nc) as tc:
        with (
            tc.tile_pool(name="const", bufs=1) as cp,
            tc.tile_pool(name="sbuf", bufs=3) as sp,
            tc.tile_pool(name="psum", bufs=1, space="PSUM") as pp,
        ):
            ident = cp.tile([128, 128], BF16)
            make_identity(nc, ident[:])
            repl = cp.tile([H, 2, 128], BF16)
            for c in range(2):
                nc.sync.dma_start(out=repl[:, c, :],
                                  in_=repl_row[:, c * 128:(c + 1) * 128])
            if use_bias:
                ones1 = cp.tile([1, 128], BF16)
                nc.gpsimd.memset(ones1[:], 1.0)
            epsd = cp.tile([H, 1], F32)
            nc.gpsimd.memset(epsd[:], 1e-30)
            epsv = cp.tile([128, 1], F32)
            nc.gpsimd.memset(epsv[:], EPS)
            # prime the activation-table set before the loops so the
            # in-loop table-load check can hoist (preheader = loaded)
            warm = cp.tile([1, 1], F32)
            nc.scalar.activation(warm[:], epsv[0:1, :], AF.Exp)

            for rel in rels:
                # --- static per-relation weights
                wq = cp.tile([128, 2, D], BF16, tag="wq")
                wk = cp.tile([128, 2, D], BF16, tag="wk")
                wv = cp.tile([128, 2, D], BF16, tag="wv")
                wmsg = cp.tile([128, 2, D], BF16, tag="wmsg")
                wskip = cp.tile([128, 2, D], BF16, tag="wskip")
                for c in range(2):
                    nc.sync.dma_start(out=wq[:, c, :], in_=rel["wq"][c * 128:(c + 1) * 128, :])
                    nc.sync.dma_start(out=wk[:, c, :], in_=rel["wk"][c * 128:(c + 1) * 128, :])
                    nc.sync.dma_start(out=wv[:, c, :], in_=rel["wv"][c * 128:(c + 1) * 128, :])
                    nc.sync.dma_start(out=wmsg[:, c, :], in_=rel["wmsg"][c * 128:(c + 1) * 128, :])
                    nc.sync.dma_start(out=wskip[:, c, :], in_=rel["wskip"][c * 128:(c + 1) * 128, :])
                if use_bias:
                    bskip = cp.tile([1, D], BF16, tag="bskip")
                    nc.sync.dma_start(out=bskip[:], in_=rel["bskip"][:])
                if use_affine:
                    gln = cp.tile([128, D], F32, tag="gln")
                    bln = cp.tile([128, D], F32, tag="bln")
                    nc.sync.dma_start(out=gln[:], in_=rel["gln"][:])
                    nc.sync.dma_start(out=bln[:], in_=rel["bln"][:])

                xfull, xdstT, srcd, oTd, odd, outd = (
                    rel["xfull"], rel["xdstT"], rel["src"], rel["oT"],
                    rel["od"], rel["out"])

                with tc.For_i(0, npad, 128 * UNROLL) as g0:
                  for u in range(UNROLL):
                    gg = g0 + 128 * u
                    # ---- loads (x_dst arrives pre-transposed)
                    xdT = sp.tile([128, 2, 128], BF16, tag="xdT", bufs=4)
                    nc.sync.dma_start(out=xdT[:], in_=xdstT[ds(gg, 128), :, :])
                    sidx = sp.tile([128, T], I32, tag="sidx", bufs=4)
                    nc.sync.dma_start(out=sidx[:], in_=srcd[ds(gg, 128), :])
                    xg = sp.tile([128, T, D], BF16, tag="xg", bufs=4)
                    for t in range(T):
                        nc.gpsimd.indirect_dma_start(
                            out=xg[:, t, :], out_offset=None,
                            in_=xfull[:],
                            in_offset=bass.IndirectOffsetOnAxis(
                                ap=sidx[:, t:t + 1], axis=0),
                        )

                    # ---- Q projection
                    qg_ps = pp.tile([128, D], F32, tag="qg", bufs=1)
                    for c in range(2):
                        nc.tensor.matmul(out=qg_ps[:], lhsT=xdT[:, c, :],
                                         rhs=wq[:, c, :],
                                         start=(c == 0), stop=(c == 1))
                    qg = sp.tile([128, D], BF16, tag="qg")
                    nc.scalar.copy(qg[:], qg_ps[:])

                    # ---- one-hot (edge -> dst-local), host-built, both
                    # orientations loaded directly
                    oT = sp.tile([128, T, 128], BF16, tag="oT", bufs=4)
                    nc.sync.dma_start(
                        out=oT[:],
                        in_=oTd[ds(gg, 128), :].rearrange(
                            "p (t d) -> p t d", d=128))
                    od = sp.tile([128, T, 128], BF16, tag="odsb", bufs=4)
                    nc.sync.dma_start(
                        out=od[:],
                        in_=odd[ds(gg, 128), :].rearrange(
                            "p (t d) -> p t d", d=128))

                    # ---- gathered x^T (tensor-engine transposes)
                    xgT_ps = pp.tile([128, T, 2, 128], BF16, tag="xgT")
                    for t in range(T):
                        for c in range(2):
                            nc.tensor.transpose(
                                out=xgT_ps[:, t, c, :],
                                in_=xg[:, t, c * 128:(c + 1) * 128],
                                identity=ident[:])
                    xgT = sp.tile([128, T, 2, 128], BF16, tag="xgTsb")
                    hT = T // 2
                    nc.vector.tensor_copy(xgT[:, :hT], xgT_ps[:, :hT])
                    nc.scalar.copy(xgT[:, hT:], xgT_ps[:, hT:])

                    # ---- per-edge K / q-expand / V; attention logits
                    qkm = sp.tile([128, T, D], BF16, tag="qkm")
                    k_sb = sp.tile([128, T, D], BF16, tag="k_sb")
                    wV = sp.tile([128, T, DA], BF16, tag="wV")
                    npair = (T + 1) // 2
                    for p in range(npair):
                        t0, t1 = 2 * p, min(2 * p + 2, T)
                        tw = t1 - t0
                        k_ps = pp.tile([128, 2, D], F32, tag="k")
                        qe_ps = pp.tile([128, 2, D], F32, tag="qe", bufs=2)
                        for t in range(t0, t1):
                            for c in range(2):
                                nc.tensor.matmul(
                                    out=k_ps[:, t - t0, :], lhsT=xgT[:, t, c, :],
                                    rhs=wk[:, c, :], start=(c == 0), stop=(c == 1))
                            nc.tensor.matmul(
                                out=qe_ps[:, t - t0, :], lhsT=od[:, t, :],
                                rhs=qg[:], start=True, stop=True)
                        nc.scalar.copy(k_sb[:, t0:t1, :], k_ps[:, :tw, :])
                        nc.vector.tensor_tensor(
                            out=qkm[:, t0:t1, :], in0=qe_ps[:, :tw, :],
                            in1=k_sb[:, t0:t1, :], op=OP.mult)

                    # exp'd logits land in the extra columns of wV, so the
                    # denominator aggregates in the same matmul stream
                    attn = sp.tile([128, T * H], F32, tag="attn")
                    ae = wV[:, :, D:DA]
                    for p in range(npair):
                        t0, t1 = 2 * p, min(2 * p + 2, T)
                        nc.vector.tensor_reduce(
                            out=attn[:, t0 * H:t1 * H],
                            in_=qkm[:, t0:t1].rearrange(
                                "p t (h j) -> p (t h) j", j=DH),
                            axis=mybir.AxisListType.X, op=OP.add)
                        nc.scalar.activation(
                            ae[:, t0:t1, :],
                            attn[:, t0 * H:t1 * H].rearrange(
                                "p (t h) -> p t h", h=H), AF.Exp)

                    # ---- V projection + weighted rows, in pairs
                    for p in range(npair):
                        t0, t1 = 2 * p, min(2 * p + 2, T)
                        tw = t1 - t0
                        v_ps = pp.tile([128, 2, D], F32, tag="v", bufs=1)
                        for t in range(t0, t1):
                            for c in range(2):
                                nc.tensor.matmul(
                                    out=v_ps[:, t - t0, :], lhsT=xgT[:, t, c, :],
                                    rhs=wv[:, c, :], start=(c == 0), stop=(c == 1))
                        nc.vector.tensor_tensor(
                            out=wV[:, t0:t1, 0:D].rearrange(
                                "p t (h j) -> p t h j", j=DH),
                            in0=ae[:, t0:t1, :, None].to_broadcast(
                                [128, tw, H, DH]),
                            in1=v_ps[:, :tw].rearrange(
                                "p t (h j) -> p t h j", j=DH),
                            op=OP.mult)

                    # ---- one-hot aggregation: numerator + denominator^T.
                    # One open accumulation chain per PSUM bank at a time;
                    # agg chunks and den share a bank, chains sequential.
                    accum_ps = pp.tile([128, 384], F32, tag="accum", bufs=1)
                    for c in range(2):
                        for t in range(T):
                            nc.tensor.matmul(
                                out=accum_ps[:, c * 128:(c + 1) * 128],
                                lhsT=wV[:, t, c * 128:(c + 1) * 128],
                                rhs=oT[:, t, :],
                                start=(t == 0), stop=(t == T - 1))
                    for t in range(T):
                        nc.tensor.matmul(
                            out=accum_ps[0:H, 256:384], lhsT=wV[:, t, D:DA],
                            rhs=oT[:, t, :],
                            start=(t == 0), stop=(t == T - 1))

                    # ---- 1/den^T = exp(-ln(den+eps)), expanded via repl
                    lnd = sp.tile([H, 128], F32, tag="lnd")
                    nc.scalar.activation(lnd[:], accum_ps[0:H, 256:384],
                                         AF.Ln, bias=epsd[:, :1])
                    dinvb = sp.tile([H, 128], BF16, tag="dinvb")
                    nc.scalar.activation(dinvb[:], lnd[:], AF.Exp, scale=-1.0)
                    R_ps = pp.tile([128, D], F32, tag="ry", bufs=1)
                    for c in range(2):
                        nc.tensor.matmul(out=R_ps[:, c * 128:(c + 1) * 128],
                                         lhsT=repl[:, c, :], rhs=dinvb[:],
                                         start=(c == 0), stop=(c == 1))
                    Rsb = sp.tile([128, 2, 128], BF16, tag="Rsb")
                    nc.scalar.copy(Rsb[:], R_ps[:].rearrange("p (c d) -> p c d", d=128))
                    aggT = sp.tile([128, 2, 128], BF16, tag="aggT")
                    nc.vector.tensor_tensor(
                        out=aggT[:],
                        in0=accum_ps[:, 0:256].rearrange("p (c d) -> p c d", d=128),
                        in1=Rsb[:], op=OP.mult)

                    # ---- y = aggT.T @ Wmsg + x @ Wskip (+ bskip); relu; LN
                    y_ps = pp.tile([128, D], F32, tag="ry", bufs=1)
                    if use_bias:
                        nc.tensor.matmul(out=y_ps[:], lhsT=ones1[:], rhs=bskip[:],
                                         start=True, stop=False)
                    for c in range(2):
                        nc.tensor.matmul(out=y_ps[:], lhsT=aggT[:, c, :],
                                         rhs=wmsg[:, c, :],
                                         start=(not use_bias and c == 0),
                                         stop=False)
                    for c in range(2):
                        nc.tensor.matmul(out=y_ps[:], lhsT=xdT[:, c, :],
                                         rhs=wskip[:, c, :], start=False,
                                         stop=(c == 1))
                    zr = sp.tile([128, D], F32, tag="zr")
                    msum = sp.tile([128, 1], F32, tag="msum")
                    nc.scalar.activation(zr[:], y_ps[:], AF.Relu,
                                         accum_out=msum[:, :1])
                    negm = sp.tile([128, 1], F32, tag="negm")
                    nc.vector.tensor_scalar(out=negm[:], in0=msum[:],
                                            scalar1=-1.0 / D, scalar2=None,
                                            op0=OP.mult)
                    sqd = sp.tile([128, D], F32, tag="sqd")
                    vs = sp.tile([128, 1], F32, tag="vs")
                    nc.scalar.activation(sqd[:], zr[:], AF.Square,
                                         bias=negm[:, :1],
                                         accum_out=vs[:, :1])
                    lnv = sp.tile([128, 1], F32, tag="lnv")
                    nc.scalar.activation(lnv[:], vs[:], AF.Ln,
                                         scale=1.0 / D, bias=epsv[:, :1])
                    rstd = sp.tile([128, 1], F32, tag="rstd")
                    nc.scalar.activation(rstd[:], lnv[:], AF.Exp, scale=-0.5)
                    fin = sp.tile([128, D], F32, tag="fin")
                    nc.vector.tensor_scalar(out=fin[:], in0=zr[:],
                                            scalar1=negm[:, :1],
                                            scalar2=rstd[:, :1],
                                            op0=OP.add, op1=OP.mult)
                    if use_affine:
                        xg2 = sp.tile([128, D], F32, tag="xg2")
                        nc.vector.tensor_tensor(out=xg2[:], in0=fin[:],
                                                in1=gln[:], op=OP.mult)
                        nc.vector.tensor_tensor(out=fin[:], in0=xg2[:],
                                                in1=bln[:], op=OP.add)
                    nc.scalar.dma_start(out=outd[ds(gg, 128), :], in_=fin[:])
    orig = bacc.get_activation_tables
    bacc.get_activation_tables = _patched_act_tables(orig)
    try:
        nc.compile()
    finally:
        bacc.get_activation_tables = orig
    return nc


# ------------------------------------------------------------------- driver

def _sigmoid(x):
    return 1.0 / (1.0 + np.exp(-x))


TRACE = False
LAST = None


def kernel(x_a, x_b, Wq_a, Wk_a, Wv_a, Wq_b, Wk_b, Wv_b,
           Wskip_a_w, Wskip_a_b, Wskip_b_w, Wskip_b_b,
           g_a, b_a, g_b, b_b, mu_ab, Wmsg_ab, mu_ba, Wmsg_ba,
           ei_ab, ei_ba):
    from concourse.bass_utils import run_bass_kernel_spmd

    x_a = np.asarray(x_a, np.float32)
    x_b = np.asarray(x_b, np.float32)
    SCALE = DH ** -0.5

    cap = max(_edge_capacity(np.asarray(ei_ab[1])),
              _edge_capacity(np.asarray(ei_ba[1])))
    T = max(2, -(-cap // 128))

    src_ab, oT_ab, od_ab = _pack_edges(
        np.asarray(ei_ab[0]), np.asarray(ei_ab[1]), T)
    src_ba, oT_ba, od_ba = _pack_edges(
        np.asarray(ei_ba[0]), np.asarray(ei_ba[1]), T)

    xa_dstT = _shard_rows_T(x_a)
    xb_dstT = _shard_rows_T(x_b)

    use_affine = not (
        np.allclose(np.asarray(g_a), 1.0) and np.allclose(np.asarray(b_a), 0.0)
        and np.allclose(np.asarray(g_b), 1.0) and np.allclose(np.asarray(b_b), 0.0))
    use_bias = bool(np.any(np.asarray(Wskip_a_b)) or np.any(np.asarray(Wskip_b_b)))

    def fold_q(Wq, mu):
        s = (SCALE * _sigmoid(np.asarray(mu, np.float64))).astype(np.float32)
        return (np.asarray(Wq, np.float32) * np.repeat(s, DH)[None, :]).astype(NPBF)

    bf = lambda v: np.asarray(v, np.float32).astype(NPBF)
    bc = lambda v: np.broadcast_to(np.asarray(v, np.float32)[None, :], (128, D)).copy()
    repl_row = (np.arange(D)[None, :] // DH ==
                np.arange(H)[:, None]).astype(NPBF).copy()

    shared = {
        "xa_full": x_a.astype(NPBF), "xb_full": x_b.astype(NPBF),
        "repl_row": repl_row,
        # relation ab: src a -> dst b (out_b)
        "wq_ab": fold_q(Wq_b, mu_ab), "wk_ab": bf(Wk_a), "wv_ab": bf(Wv_a),
        "wmsg_ab": bf(Wmsg_ab), "wskip_ab": bf(Wskip_b_w),
        # relation ba: src b -> dst a (out_a)
        "wq_ba": fold_q(Wq_a, mu_ba), "wk_ba": bf(Wk_b), "wv_ba": bf(Wv_b),
        "wmsg_ba": bf(Wmsg_ba), "wskip_ba": bf(Wskip_a_w),
    }
    if use_bias:
        shared["bskip_ab"] = bf(Wskip_b_b).reshape(1, D)
        shared["bskip_ba"] = bf(Wskip_a_b).reshape(1, D)
    if use_affine:
        shared["gln_ab"] = bc(g_b)
        shared["bln_ab"] = bc(b_b)
        shared["gln_ba"] = bc(g_a)
        shared["bln_ba"] = bc(b_a)

    in_maps = []
    for m in range(M):
        im = dict(shared)
        im["xa_dstT"] = xa_dstT[m]
        im["xb_dstT"] = xb_dstT[m]
        im["src_ab"] = src_ab[m]
        im["oT_ab"] = oT_ab[m]
        im["od_ab"] = od_ab[m]
        im["src_ba"] = src_ba[m]
        im["oT_ba"] = oT_ba[m]
        im["od_ba"] = od_ba[m]
        in_maps.append(im)

    nc = build_program(T, use_affine, use_bias)
    res = run_bass_kernel_spmd(nc, in_maps, list(range(M)), trace=TRACE)
    global LAST
    LAST = res
    out_a = np.empty((N, D), np.float32)
    out_b = np.empty((N, D), np.float32)
    for m in range(M):
        out_b[m * NSH:(m + 1) * NSH] = res.results[m]["out_ab"][:NSH]
        out_a[m * NSH:(m + 1) * NSH] = res.results[m]["out_ba"][:NSH]
    return out_a, out_b


# revision 32
# speedup vs baseline: 1.7607x; 1.3949x over previous
"""HGT layer (2 node types, 2 relations) on 8 Trainium2 cores.

Strategy (dst-sharded, fully fused single pass):
  - Each core owns 12500 destination nodes of each type. Edges are
    partitioned by destination shard on the host and sorted into groups
    of 128 consecutive destination nodes, padded to T*128 slots.
  - bf16 datapath: gathers, one-hot matrices, and every matmul operand
    are bf16 (PE runs 1 cycle/row vs 4 for fp32; gather HBM traffic
    halves).  PSUM accumulation and the layernorm chain stay fp32.
  - x_dst arrives pre-transposed from the host (direct lhsT for the
    Q/skip matmuls); the per-edge one-hot matrices are also host-built
    in both orientations and simply DMA-loaded, so the vector engine
    never constructs them.  Gathered source rows are transposed on the
    tensor engine (128x128 bf16 transposes).
  - The scalar engine only ever uses {Copy, Relu, Square, Exp, Ln} —
    all within the single `natural_log_exp_and_others` activation-table
    set (the compile patches the table map so the load-inserter can
    prove this; a pre-loop warm activation makes the in-loop load
    hoistable) — so no ACT_TABLE_LOAD reloads occur in steady state.
    rsqrt (layernorm) and the attention-denominator reciprocal are both
    computed as exp(-a*ln(x)) on the scalar engine.
  - The per-edge exp'd attention logits are written into 8 extra
    columns appended to the weighted-V tile, so the softmax denominator
    falls out of the same one-hot aggregation matmul stream as the
    numerator (transposed: den^T [h, d]); a tiny one-hot head-replication
    matmul expands 1/den back to feature rows.
  - Only one PSUM accumulation chain is ever open per PSUM bank, and a
    chain's matmuls are always consecutive on the tensor engine.
  - The group loop is unrolled 49x inside tc.For_i: the per-iteration
    all-engine barrier amortizes away and the Tile scheduler pipelines
    the unrolled groups across engines (PSUM tag bufs are chosen so
    cross-group waits land on weak positions of the dependency chain).
  - The per-head attention scale (SCALE * sigmoid(mu_h)) is folded into
    Wq on the host.  When the LN affine params are identity and the skip
    bias is zero (always true for this model's inputs), those ops are
    compiled out.
"""

import numpy as np
import ml_dtypes

import concourse.bacc as bacc
import concourse.bass as bass
import concourse.mybir as mybir
import concourse.tile as tile
from concourse.bass import ds
from concourse.masks import make_identity

N = 100000
D = 256
H = 8
DH = 32
M = 8            # cores
NSH = N // M     # 12500 dst rows per core per type
G = 98           # dst groups of 128 per core (98*128 = 12544)
NPAD = G * 128   # 12544
UNROLL = 49      # dst groups per hardware-loop iteration
EPS = 1e-5
DA = D + H       # wV tile width with appended exp'd logits
F32 = mybir.dt.float32
BF16 = mybir.dt.bfloat16
I32 = mybir.dt.int32
AF = mybir.ActivationFunctionType
OP = mybir.AluOpType
NPBF = ml_dtypes.bfloat16

ACT_SET = "natural_log_exp_and_others"
ACT_FNS = {AF.Copy, AF.Exp, AF.Relu, AF.Square, AF.Ln}


# ----------------------------------------------------------------- host prep

def _pack_edges(src, dst, T):
    """Partition edges by dst shard, group by 128 consecutive dsts, pad to
    T*128 slots per group.  Returns src_idx [M, NPAD, T] int32 and
    dstl [M, NPAD, T] bf16 (dst-local-in-group; 999.0 for padding).
    Slot s of group g maps to SBUF (partition p = s % 128, column t = s // 128),
    i.e. row g*128 + p, col t of the packed array."""
    order = np.argsort(dst, kind="stable")
    s_sorted = src[order].astype(np.int64)
    d_sorted = dst[order].astype(np.int64)

    core = d_sorted // NSH
    local = d_sorted - core * NSH
    grp = local // 128
    dloc = local - grp * 128
    key = core * G + grp
    # rank of each edge within its (core, group)
    first = np.r_[0, np.flatnonzero(np.diff(key)) + 1]
    starts = np.zeros(len(key), dtype=np.int64)
    starts[first] = first
    starts = np.maximum.accumulate(starts)
    slot = np.arange(len(key), dtype=np.int64) - starts

    maxslot = int(slot.max()) if len(slot) else 0
    assert maxslot < T * 128, f"edge capacity exceeded: {maxslot + 1} > {T * 128}"

    src_arr = np.zeros((M * G, T * 128), dtype=np.int32)
    dst_arr = np.full((M * G, T * 128), 999.0, dtype=np.float32)
    src_arr[key, slot] = s_sorted
    dst_arr[key, slot] = dloc
    # [MG, T, 128] -> [MG, 128, T] -> [M, NPAD, T]
    src_arr = src_arr.reshape(M * G, T, 128).transpose(0, 2, 1)
    dst_arr = dst_arr.reshape(M * G, T, 128).transpose(0, 2, 1)  # [MG, e, T]
    # one-hot matrices in both orientations, built host-side:
    #   oT[e, t, d] = (dstl[e, t] == d)   (edge-major, rhs/lhsT of aggregation)
    #   od[d, t, e] = (dstl[e, t] == d)   (dst-major, lhsT of q-expansion)
    eq = dst_arr[:, :, :, None] == np.arange(128, dtype=np.float32)
    oT_h = eq.astype(NPBF).reshape(M, NPAD, T * 128)
    od_h = eq.transpose(0, 3, 2, 1).astype(NPBF).reshape(M, NPAD, T * 128)
    return (src_arr.reshape(M, NPAD, T).copy(),
            oT_h.copy(), od_h.copy())


def _edge_capacity(dst):
    d = np.sort(dst.astype(np.int64))
    core = d // NSH
    grp = (d - core * NSH) // 128
    key = core * G + grp
    _, counts = np.unique(key, return_counts=True)
    return int(counts.max())


def _shard_rows_T(x):
    """[N, D] -> [M, NPAD, 2, 128] bf16, zero padded, per-group transposed:
    out[m, g*128 + p, c, e] = x[m*NSH + g*128 + e, c*128 + p]."""
    out = np.zeros((M, G, 128, 2, 128), dtype=NPBF)
    for m in range(M):
        blk = np.zeros((NPAD, D), dtype=np.float32)
        blk[:NSH] = x[m * NSH:(m + 1) * NSH]
        # [G, 128e, 2c, 128p] -> [G, p, c, e]
        out[m] = blk.reshape(G, 128, 2, 128).transpose(0, 3, 2, 1).astype(NPBF)
    return out.reshape(M, NPAD, 2, 128)


# ------------------------------------------------------------- bass program

def _patched_act_tables(orig_fn):
    """Return a wrapper that strips this kernel's activation functions from
    every table set except ACT_SET, so the table-load inserter maps them all
    to one set (positions/indices of the sets are preserved)."""
    def wrapper(arch):
        t = orig_fn(arch)
        if ACT_SET in t:
            for name in t:
                if name != ACT_SET:
                    t[name] = t[name] - ACT_FNS
        return t
    return wrapper


def build_program(T, use_affine, use_bias, nfull=N, npad=NPAD):
    nc = bacc.Bacc("TRN2", target_bir_lowering=False, debug=False)

    def drt(name, shape, dtype=F32, kind="ExternalInput"):
        return nc.dram_tensor(name, shape, dtype, kind=kind)

    xa_full = drt("xa_full", [nfull, D], BF16)
    xb_full = drt("xb_full", [nfull, D], BF16)
    xa_dstT = drt("xa_dstT", [npad, 2, 128], BF16)
    xb_dstT = drt("xb_dstT", [npad, 2, 128], BF16)
    repl_row = drt("repl_row", [H, D], BF16)    # repl[h, f] = (f//DH == h)

    rels = []
    for r in ("ab", "ba"):
        rel = dict(
            name=r,
            src=drt(f"src_{r}", [npad, T], I32),
            oT=drt(f"oT_{r}", [npad, T * 128], BF16),
            od=drt(f"od_{r}", [npad, T * 128], BF16),
            wq=drt(f"wq_{r}", [D, D], BF16),
            wk=drt(f"wk_{r}", [D, D], BF16),
            wv=drt(f"wv_{r}", [D, D], BF16),
            wmsg=drt(f"wmsg_{r}", [D, D], BF16),
            wskip=drt(f"wskip_{r}", [D, D], BF16),
            out=drt(f"out_{r}", [npad, D], kind="ExternalOutput"),
        )
        if use_bias:
            rel["bskip"] = drt(f"bskip_{r}", [1, D], BF16)
        if use_affine:
            rel["gln"] = drt(f"gln_{r}", [128, D])
            rel["bln"] = drt(f"bln_{r}", [128, D])
        rels.append(rel)
    rels[0]["xfull"] = xa_full   # ab: src type a
    rels[0]["xdstT"] = xb_dstT   # ab: dst type b
    rels[1]["xfull"] = xb_full
    rels[1]["xdstT"] = xa_dstT

    with tile.TileContext(nc) as tc:
        with (
            tc.tile_pool(name="const", bufs=1) as cp,
            tc.tile_pool(name="sbuf", bufs=4) as sp,
            tc.tile_pool(name="psum", bufs=1, space="PSUM") as pp,
        ):
            ident = cp.tile([128, 128], BF16)
            make_identity(nc, ident[:])
            repl = cp.tile([H, 2, 128], BF16)
            for c in range(2):
                nc.sync.dma_start(out=repl[:, c, :],
                                  in_=repl_row[:, c * 128:(c + 1) * 128])
            if use_bias:
                ones1 = cp.tile([1, 128], BF16)
                nc.gpsimd.memset(ones1[:], 1.0)
            epsd = cp.tile([H, 1], F32)
            nc.gpsimd.memset(epsd[:], 1e-30)
            epsv = cp.tile([128, 1], F32)
            nc.gpsimd.memset(epsv[:], EPS)
            # prime the activation-table set before the loops so the
            # in-loop table-load check can hoist (preheader = loaded)
            warm = cp.tile([1, 1], F32)
            nc.scalar.activation(warm[:], epsv[0:1, :], AF.Exp)

            for rel in rels:
                # --- static per-relation weights
                wq = cp.tile([128, 2, D], BF16, tag="wq")
                wk = cp.tile([128, 2, D], BF16, tag="wk")
                wv = cp.tile([128, 2, D], BF16, tag="wv")
                wmsg = cp.tile([128, 2, D], BF16, tag="wmsg")
                wskip = cp.tile([128, 2, D], BF16, tag="wskip")
                for c in range(2):
                    nc.sync.dma_start(out=wq[:, c, :], in_=rel["wq"][c * 128:(c + 1) * 128, :])
                    nc.sync.dma_start(out=wk[:, c, :], in_=rel["wk"][c * 128:(c + 1) * 128, :])
                    nc.sync.dma_start(out=wv[:, c, :], in_=rel["wv"][c * 128:(c + 1) * 128, :])
                    nc.sync.dma_start(out=wmsg[:, c, :], in_=rel["wmsg"][c * 128:(c + 1) * 128, :])
                    nc.sync.dma_start(out=wskip[:, c, :], in_=rel["wskip"][c * 128:(c + 1) * 128, :])
                if use_bias:
                    bskip = cp.tile([1, D], BF16, tag="bskip")
                    nc.sync.dma_start(out=bskip[:], in_=rel["bskip"][:])
                if use_affine:
                    gln = cp.tile([128, D], F32, tag="gln")
                    bln = cp.tile([128, D], F32, tag="bln")
                    nc.sync.dma_start(out=gln[:], in_=rel["gln"][:])
                    nc.sync.dma_start(out=bln[:], in_=rel["bln"][:])

                xfull, xdstT, srcd, oTd, odd, outd = (
                    rel["xfull"], rel["xdstT"], rel["src"], rel["oT"],
                    rel["od"], rel["out"])

                with tc.For_i(0, npad, 128 * UNROLL) as g0:
                  for u in range(UNROLL):
                    gg = g0 + 128 * u
                    # ---- loads (x_dst arrives pre-transposed)
                    xdT = sp.tile([128, 2, 128], BF16, tag="xdT", bufs=6)
                    nc.sync.dma_start(out=xdT[:], in_=xdstT[ds(gg, 128), :, :])
                    sidx = sp.tile([128, T], I32, tag="sidx", bufs=6)
                    nc.sync.dma_start(out=sidx[:], in_=srcd[ds(gg, 128), :])
                    xg = sp.tile([128, T, D], BF16, tag="xg", bufs=6)
                    for t in range(T):
                        nc.gpsimd.indirect_dma_start(
                            out=xg[:, t, :], out_offset=None,
                            in_=xfull[:],
                            in_offset=bass.IndirectOffsetOnAxis(
                                ap=sidx[:, t:t + 1], axis=0),
                        )

                    # ---- Q projection
                    qg_ps = pp.tile([128, D], F32, tag="qg", bufs=1)
                    for c in range(2):
                        nc.tensor.matmul(out=qg_ps[:], lhsT=xdT[:, c, :],
                                         rhs=wq[:, c, :],
                                         start=(c == 0), stop=(c == 1))
                    qg = sp.tile([128, D], BF16, tag="qg")
                    nc.scalar.copy(qg[:], qg_ps[:])

                    # ---- one-hot (edge -> dst-local), host-built, both
                    # orientations loaded directly
                    oT = sp.tile([128, T, 128], BF16, tag="oT", bufs=6)
                    nc.sync.dma_start(
                        out=oT[:],
                        in_=oTd[ds(gg, 128), :].rearrange(
                            "p (t d) -> p t d", d=128))
                    od = sp.tile([128, T, 128], BF16, tag="odsb", bufs=6)
                    nc.sync.dma_start(
                        out=od[:],
                        in_=odd[ds(gg, 128), :].rearrange(
                            "p (t d) -> p t d", d=128))

                    # ---- gathered x^T (tensor-engine transposes)
                    xgT_ps = pp.tile([128, T, 2, 128], BF16, tag="xgT")
                    for t in range(T):
                        for c in range(2):
                            nc.tensor.transpose(
                                out=xgT_ps[:, t, c, :],
                                in_=xg[:, t, c * 128:(c + 1) * 128],
                                identity=ident[:])
                    xgT = sp.tile([128, T, 2, 128], BF16, tag="xgTsb")
                    hT = T // 2
                    nc.vector.tensor_copy(xgT[:, :hT], xgT_ps[:, :hT])
                    nc.scalar.copy(xgT[:, hT:], xgT_ps[:, hT:])

                    # ---- per-edge K / q-expand / V; attention logits
                    qkm = sp.tile([128, T, D], BF16, tag="qkm")
                    k_sb = sp.tile([128, T, D], BF16, tag="k_sb")
                    wV = sp.tile([128, T, DA], BF16, tag="wV")
                    npair = (T + 1) // 2
                    for p in range(npair):
                        t0, t1 = 2 * p, min(2 * p + 2, T)
                        tw = t1 - t0
                        k_ps = pp.tile([128, 2, D], F32, tag="k")
                        qe_ps = pp.tile([128, 2, D], F32, tag="qe", bufs=2)
                        for t in range(t0, t1):
                            for c in range(2):
                                nc.tensor.matmul(
                                    out=k_ps[:, t - t0, :], lhsT=xgT[:, t, c, :],
                                    rhs=wk[:, c, :], start=(c == 0), stop=(c == 1))
                            nc.tensor.matmul(
                                out=qe_ps[:, t - t0, :], lhsT=od[:, t, :],
                                rhs=qg[:], start=True, stop=True)
                        nc.scalar.copy(k_sb[:, t0:t1, :], k_ps[:, :tw, :])
                        nc.vector.tensor_tensor(
                            out=qkm[:, t0:t1, :], in0=qe_ps[:, :tw, :],
                            in1=k_sb[:, t0:t1, :], op=OP.mult)

                    # exp'd logits land in the extra columns of wV, so the
                    # denominator aggregates in the same matmul stream
                    attn = sp.tile([128, T * H], F32, tag="attn")
                    ae = wV[:, :, D:DA]
                    for p in range(npair):
                        t0, t1 = 2 * p, min(2 * p + 2, T)
                        nc.vector.tensor_reduce(
                            out=attn[:, t0 * H:t1 * H],
                            in_=qkm[:, t0:t1].rearrange(
                                "p t (h j) -> p (t h) j", j=DH),
                            axis=mybir.AxisListType.X, op=OP.add)
                        nc.scalar.activation(
                            ae[:, t0:t1, :],
                            attn[:, t0 * H:t1 * H].rearrange(
                                "p (t h) -> p t h", h=H), AF.Exp)

                    # ---- V projection + weighted rows, in pairs
                    for p in range(npair):
                        t0, t1 = 2 * p, min(2 * p + 2, T)
                        tw = t1 - t0
                        v_ps = pp.tile([128, 2, D], F32, tag="v", bufs=1)
                        for t in range(t0, t1):
                            for c in range(2):
                                nc.tensor.matmul(
                                    out=v_ps[:, t - t0, :], lhsT=xgT[:, t, c, :],
                                    rhs=wv[:, c, :], start=(c == 0), stop=(c == 1))
                        nc.vector.tensor_tensor(
                            out=wV[:, t0:t1, 0:D].rearrange(
                                "p t (h j) -> p t h j", j=DH),
                            in0=ae[:, t0:t1, :, None].to_broadcast(
                                [128, tw, H, DH]),
                            in1=v_ps[:, :tw].rearrange(
                                "p t (h j) -> p t h j", j=DH),
                            op=OP.mult)

                    # ---- one-hot aggregation: numerator + denominator^T.
                    # One open accumulation chain per PSUM bank at a time;
                    # agg chunks and den share a bank, chains sequential.
                    accum_ps = pp.tile([128, 384], F32, tag="accum", bufs=1)
                    for c in range(2):
                        for t in range(T):
                            nc.tensor.matmul(
                                out=accum_ps[:, c * 128:(c + 1) * 128],
                                lhsT=wV[:, t, c * 128:(c + 1) * 128],
                                rhs=oT[:, t, :],
                                start=(t == 0), stop=(t == T - 1))
                    for t in range(T):
                        nc.tensor.matmul(
                            out=accum_ps[0:H, 256:384], lhsT=wV[:, t, D:DA],
                            rhs=oT[:, t, :],
                            start=(t == 0), stop=(t == T - 1))

                    # ---- 1/den^T = exp(-ln(den+eps)), expanded via repl
                    lnd = sp.tile([H, 128], F32, tag="lnd")
                    nc.scalar.activation(lnd[:], accum_ps[0:H, 256:384],
                                         AF.Ln, bias=epsd[:, :1])
                    dinvb = sp.tile([H, 128], BF16, tag="dinvb")
                    nc.scalar.activation(dinvb[:], lnd[:], AF.Exp, scale=-1.0)
                    R_ps = pp.tile([128, D], F32, tag="ry", bufs=1)
                    for c in range(2):
                        nc.tensor.matmul(out=R_ps[:, c * 128:(c + 1) * 128],
                                         lhsT=repl[:, c, :], rhs=dinvb[:],
                                         start=(c == 0), stop=(c == 1))
                    Rsb = sp.tile([128, 2, 128], BF16, tag="Rsb")
                    nc.scalar.copy(Rsb[:], R_ps[:].rearrange("p (c d) -> p c d", d=128))
                    aggT = sp.tile([128, 2, 128], BF16, tag="aggT")
                    nc.vector.tensor_tensor(
                        out=aggT[:],
                        in0=accum_ps[:, 0:256].rearrange("p (c d) -> p c d", d=128),
                        in1=Rsb[:], op=OP.mult)

                    # ---- y = aggT.T @ Wmsg + x @ Wskip (+ bskip); relu; LN
                    y_ps = pp.tile([128, D], F32, tag="ry", bufs=1)
                    if use_bias:
                        nc.tensor.matmul(out=y_ps[:], lhsT=ones1[:], rhs=bskip[:],
                                         start=True, stop=False)
                    for c in range(2):
                        nc.tensor.matmul(out=y_ps[:], lhsT=aggT[:, c, :],
                                         rhs=wmsg[:, c, :],
                                         start=(not use_bias and c == 0),
                                         stop=False)
                    for c in range(2):
                        nc.tensor.matmul(out=y_ps[:], lhsT=xdT[:, c, :],
                                         rhs=wskip[:, c, :], start=False,
                                         stop=(c == 1))
                    zr = sp.tile([128, D], F32, tag="zr")
                    msum = sp.tile([128, 1], F32, tag="msum")
                    nc.scalar.activation(zr[:], y_ps[:], AF.Relu,
                                         accum_out=msum[:, :1])
                    negm = sp.tile([128, 1], F32, tag="negm")
                    nc.vector.tensor_scalar(out=negm[:], in0=msum[:],
                                            scalar1=-1.0 / D, scalar2=None,
                                            op0=OP.mult)
                    sqd = sp.tile([128, D], F32, tag="sqd")
                    vs = sp.tile([128, 1], F32, tag="vs")
                    nc.scalar.activation(sqd[:], zr[:], AF.Square,
                                         bias=negm[:, :1],
                                         accum_out=vs[:, :1])
                    lnv = sp.tile([128, 1], F32, tag="lnv")
                    nc.scalar.activation(lnv[:], vs[:], AF.Ln,
                                         scale=1.0 / D, bias=epsv[:, :1])
                    rstd = sp.tile([128, 1], F32, tag="rstd")
                    nc.scalar.activation(rstd[:], lnv[:], AF.Exp, scale=-0.5)
                    fin = sp.tile([128, D], F32, tag="fin")
                    nc.vector.tensor_scalar(out=fin[:], in0=zr[:],
                                            scalar1=negm[:, :1],
                                            scalar2=rstd[:, :1],
                                            op0=OP.add, op1=OP.mult)
                    if use_affine:
                        xg2 = sp.tile([128, D], F32, tag="xg2")
                        nc.vector.tensor_tensor(out=xg2[:], in0=fin[:],
                                                in1=gln[:], op=OP.mult)
                        nc.vector.tensor_tensor(out=fin[:], in0=xg2[:],
                                                in1=bln[:], op=OP.add)
                    nc.scalar.dma_start(out=outd[ds(gg, 128), :], in_=fin[:])
    orig = bacc.get_activation_tables
    bacc.get_activation_tables = _patched_act_tables(orig)
    try:
        nc.compile()
    finally:
        bacc.get_activation_tables = orig
    return nc


# ------------------------------------------------------------------- driver

def _sigmoid(x):
    return 1.0 / (1.0 + np.exp(-x))


TRACE = False
LAST = None


def kernel(x_a, x_b, Wq_a, Wk_a, Wv_a, Wq_b, Wk_b, Wv_b,
           Wskip_a_w, Wskip_a_b, Wskip_b_w, Wskip_b_b,
           g_a, b_a, g_b, b_b, mu_ab, Wmsg_ab, mu_ba, Wmsg_ba,
           ei_ab, ei_ba):
    from concourse.bass_utils import run_bass_kernel_spmd

    x_a = np.asarray(x_a, np.float32)
    x_b = np.asarray(x_b, np.float32)
    SCALE = DH ** -0.5

    cap = max(_edge_capacity(np.asarray(ei_ab[1])),
              _edge_capacity(np.asarray(ei_ba[1])))
    T = max(2, -(-cap // 128))

    src_ab, oT_ab, od_ab = _pack_edges(
        np.asarray(ei_ab[0]), np.asarray(ei_ab[1]), T)
    src_ba, oT_ba, od_ba = _pack_edges(
        np.asarray(ei_ba[0]), np.asarray(ei_ba[1]), T)

    xa_dstT = _shard_rows_T(x_a)
    xb_dstT = _shard_rows_T(x_b)

    use_affine = not (
        np.allclose(np.asarray(g_a), 1.0) and np.allclose(np.asarray(b_a), 0.0)
        and np.allclose(np.asarray(g_b), 1.0) and np.allclose(np.asarray(b_b), 0.0))
    use_bias = bool(np.any(np.asarray(Wskip_a_b)) or np.any(np.asarray(Wskip_b_b)))

    def fold_q(Wq, mu):
        s = (SCALE * _sigmoid(np.asarray(mu, np.float64))).astype(np.float32)
        return (np.asarray(Wq, np.float32) * np.repeat(s, DH)[None, :]).astype(NPBF)

    bf = lambda v: np.asarray(v, np.float32).astype(NPBF)
    bc = lambda v: np.broadcast_to(np.asarray(v, np.float32)[None, :], (128, D)).copy()
    repl_row = (np.arange(D)[None, :] // DH ==
                np.arange(H)[:, None]).astype(NPBF).copy()

    shared = {
        "xa_full": x_a.astype(NPBF), "xb_full": x_b.astype(NPBF),
        "repl_row": repl_row,
        # relation ab: src a -> dst b (out_b)
        "wq_ab": fold_q(Wq_b, mu_ab), "wk_ab": bf(Wk_a), "wv_ab": bf(Wv_a),
        "wmsg_ab": bf(Wmsg_ab), "wskip_ab": bf(Wskip_b_w),
        # relation ba: src b -> dst a (out_a)
        "wq_ba": fold_q(Wq_a, mu_ba), "wk_ba": bf(Wk_b), "wv_ba": bf(Wv_b),
        "wmsg_ba": bf(Wmsg_ba), "wskip_ba": bf(Wskip_a_w),
    }
    if use_bias:
        shared["bskip_ab"] = bf(Wskip_b_b).reshape(1, D)
        shared["bskip_ba"] = bf(Wskip_a_b).reshape(1, D)
    if use_affine:
        shared["gln_ab"] = bc(g_b)
        shared["bln_ab"] = bc(b_b)
        shared["gln_ba"] = bc(g_a)
        shared["bln_ba"] = bc(b_a)

    in_maps = []
    for m in range(M):
        im = dict(shared)
        im["xa_dstT"] = xa_dstT[m]
        im["xb_dstT"] = xb_dstT[m]
        im["src_ab"] = src_ab[m]
        im["oT_ab"] = oT_ab[m]
        im["od_ab"] = od_ab[m]
        im["src_ba"] = src_ba[m]
        im["oT_ba"] = oT_ba[m]
        im["od_ba"] = od_ba[m]
        in_maps.append(im)

    nc = build_program(T, use_affine, use_bias)
    res = run_bass_kernel_spmd(nc, in_maps, list(range(M)), trace=TRACE)
    global LAST
    LAST = res
    out_a = np.empty((N, D), np.float32)
    out_b = np.empty((N, D), np.float32)
    for m in range(M):
        out_b[m * NSH:(m + 1) * NSH] = res.results[m]["out_ab"][:NSH]
        out_a[m * NSH:(m + 1) * NSH] = res.results[m]["out_ba"][:NSH]
    return out_a, out_b
